# revision 12
# baseline (speedup 1.0000x reference)
"""Trainium2 Bass kernel for nn_CBlock3D: Conv3d(16->32, k=3, SAME) + BatchNorm3d
(training-mode batch stats) + softplus, on x[4,16,16,64,64] f32.

Strategy (8 NeuronCores, SPMD), v2:
  - Shard (batch n, depth-half dh): 8 shards of [16, 8, 64, 64] output depth-slabs.
    Host uploads the raw padded volume xs[16, XBLEN] fp16 once; on device each
    plane is HBM-loaded once ([16, PL+2]) and the kw=1/2 shifted partition
    copies are built by SBUF->SBUF DMA (HBM-in traffic 1.4MB vs 4.2MB).
  - PE 2x4 tiling (64x32 mode): even-index planes live in SBUF partitions
    0-47 (kw-triplicated (kw,ci) rows), odd planes in partitions 64-111.
    The two 64-row PE row-groups run concurrently: for a slab pair
    (d0=2m, d1=2m+1), the even row-half processes d0's kd in {0,2} taps
    (6) then d1's kd=1 taps (3), while the odd half processes d1's kd in
    {0,2} (6) then d0's kd=1 (3). Both halves stream 9 taps x 2 hh x 512
    cols; per-bank ownership switches between halves only at the round-6
    boundary with a full 213ns matmul slot of separation (PSUM banks must
    never see two row-groups simultaneously). 96/128 PE rows busy.
  - Per tap: 8 matmuls (2 hh x 4 col strips of 8 h-rows), K=48, N=512,
    accumulating into psum bank (d, hh); start on the bank's first tap,
    stop on its 9th.
  - Evacuation: ScalarE copies psum -> y_all (fp16), VectorE bn_stats.
  - bn_aggr -> AllReduce[128,2] -> mean/istd -> per-partition scale/shift
    (identical to v1).
  - Phase 2: native Softplus activation in one ScalarE pass (in-place on
    fp16 y_all), chunked and overlapped with fp16 output DMA.
"""

import numpy as np
from contextlib import ExitStack

import concourse.bacc as bacc
import concourse.bass as bass
import concourse.tile as tile
from concourse import mybir

N, CIN, COUT, KK = 4, 16, 32, 3
D, H, W = 16, 64, 64
NCORES = 8
DSH = D // 2          # 8 output d-planes per core
HP, WP = H + 2, W + 2  # padded plane 66x66
PL = HP * WP           # 4356 elements per padded plane
PLP = PL + 2           # plane tile free size (kw=1/2 shift tail)
NPLANES = DSH + 2      # 10 input planes per core
NPAIRS = NPLANES // 2  # 5 plane-pair tiles
XBLEN = NPLANES * PL + 2
NTILES = DSH * 2       # (d, h-half) tiles
NCOL = 512             # psum free dim per tile
EPS = 1e-5

DT_MM = mybir.dt.float16


def _hamming(n):
    if n == 1:
        return np.ones((1,), np.float32)
    i = np.arange(n, dtype=np.float32)
    return (0.54 - 0.46 * np.cos(2.0 * np.float32(np.pi) * i / (n - 1))).astype(
        np.float32
    )


def preprocess_weights(weight):
    """shrink_conv_weights + hamming window, all fp32 numpy (matches reference)."""
    w = weight.astype(np.float32)
    cutoff = w.max(axis=(2, 3, 4), keepdims=True) * np.float32(0.5)
    shrunk = np.sign(w) * np.maximum(np.abs(w) - cutoff / np.float32(100.0), 0.0)
    w = np.where(w < cutoff, shrunk, w)
    win = (
        _hamming(KK)[:, None, None]
        * _hamming(KK)[None, :, None]
        * _hamming(KK)[None, None, :]
    )
    return (w * win[None, None]).astype(np.float32)


def build_w9(w):
    """w [COUT, CIN, 3,3,3] -> [9, 48, 32]: W9[kd*3+kh, kw*16+ci, co]."""
    w9 = np.transpose(w, (2, 3, 4, 1, 0))  # [kd, kh, kw, ci, co]
    return np.ascontiguousarray(w9.reshape(9, KK * CIN, COUT))


def build_xs(x_shard_padded):
    """[16, 10, 66, 66] fp32 -> [16, XBLEN] fp16 flat padded volume (+2 tail)."""
    xf = x_shard_padded.reshape(CIN, -1)  # [16, 43560]
    xs = np.zeros((CIN, XBLEN), np.float16)
    xs[:, : xf.shape[1]] = xf
    return xs


def build_program(r1=1, rc=1, r2=1, cc_eng="gpsimd", rall=1,
                  psum_bufs=8, x_bufs=5, ch_tiles=4, softplus=False,
                  sbuf_shift=1):
    nc = bacc.Bacc(None, target_bir_lowering=False)
    CC_ENG = getattr(nc, cc_eng)
    xs_d = nc.dram_tensor("xs", [CIN, XBLEN], DT_MM, kind="ExternalInput")
    w9_d = nc.dram_tensor("w9", [9, KK * CIN, COUT], DT_MM, kind="ExternalInput")
    gb_d = nc.dram_tensor("gb", [2, COUT], mybir.dt.float32, kind="ExternalInput")
    y_d = nc.dram_tensor("y", [DSH, 2, 4, COUT, 8, W], DT_MM, kind="ExternalOutput")

    f32 = mybir.dt.float32
    with tile.TileContext(nc) as tc:
        with ExitStack() as ctx:
            singles = ctx.enter_context(tc.tile_pool(name="singles", bufs=1))
            xpool = ctx.enter_context(tc.tile_pool(name="xpairs", bufs=x_bufs))
            psum = ctx.enter_context(
                tc.tile_pool(name="psum", bufs=psum_bufs, space="PSUM")
            )
            small = ctx.enter_context(tc.tile_pool(name="small", bufs=2))
            dram = ctx.enter_context(tc.tile_pool(name="dram", bufs=2, space="DRAM"))

            # weights, duplicated into both PE row-halves' partitions
            w_sb = singles.tile([128, 9, COUT], DT_MM)
            wsrc = w9_d[:, :, :].rearrange("r p m -> p r m")
            nc.sync.dma_start(out=w_sb[0:48, :, :], in_=wsrc)
            nc.sync.dma_start(out=w_sb[64:112, :, :], in_=wsrc)

            gb_sb = singles.tile([128, 2], f32)
            gbd = gb_d[:, :]
            for j in range(2):
                nc.sync.dma_start(
                    out=gb_sb[:, j : j + 1],
                    in_=bass.AP(
                        tensor=gbd.tensor, offset=j * COUT,
                        ap=[[0, 4], [1, COUT], [1, 1]],
                    ),
                )
            eps_sb = singles.tile([128, 1], f32)
            nc.vector.memset(eps_sb, EPS)

            y_all = singles.tile([128, NTILES * NCOL], f32)
            y16 = singles.tile([128, NTILES * NCOL], DT_MM)
            stats_all = singles.tile([128, NTILES, 6], f32)

            pairs = [None] * NPAIRS
            xs_ap = xs_d[:, :]

            def load_pair(j):
                """Pair tile j: plane 2j -> rows 0-47, plane 2j+1 -> rows 64-111.
                Rows base+0:16 = kw0 (HBM), base+16:32 = kw1, base+32:48 = kw2
                (SBUF->SBUF shifted copies)."""
                pt = xpool.tile([128, PLP], DT_MM, tag="pair", name=f"pair{j}")
                pairs[j] = pt
                for h, p in ((0, 2 * j), (64, 2 * j + 1)):
                    if sbuf_shift:
                        nc.sync.dma_start(
                            out=pt[h : h + 16, :],
                            in_=bass.AP(
                                tensor=xs_ap.tensor, offset=p * PL,
                                ap=[[XBLEN, CIN], [1, PLP]],
                            ),
                        )
                        for kw in (1, 2):
                            nc.sync.dma_start(
                                out=pt[h + 16 * kw : h + 16 * kw + 16, 0:PL],
                                in_=pt[h : h + 16, kw : kw + PL],
                            )
                    else:
                        for kw in range(KK):
                            nc.sync.dma_start(
                                out=pt[h + 16 * kw : h + 16 * kw + 16, 0:PL],
                                in_=bass.AP(
                                    tensor=xs_ap.tensor, offset=p * PL + kw,
                                    ap=[[XBLEN, CIN], [1, PL]],
                                ),
                            )

            def tap_rhs(p, kh, hh, b):
                pt = pairs[p // 2]
                base = 64 * (p % 2)
                src = pt[base : base + 48, 0:PL].rearrange(
                    "q (h w) -> q h w", w=WP
                )
                h0 = hh * 32 + b * 8 + kh
                return base, src[:, h0 : h0 + 8, 0:W]

            for _ra in range(rall):
             for _rep1 in range(r1):
              for j in range(min(2, NPAIRS)):
                load_pair(j)
              for m in range(DSH // 2):  # slab pairs (d0, d1)
                if m + 2 < NPAIRS:
                    load_pair(m + 2)
                d0, d1 = 2 * m, 2 * m + 1
                # per-half tap orders (dslab, kd, kh): 6 primary-slab taps
                # then 3 for the other slab. Each PSUM bank is written by
                # exactly ONE PE row-group (different row tiles must never
                # touch the same bank); the odd half's contribution to d0
                # (and even's to d1) goes to a separate "cross" bank, merged
                # by the DVE during evacuation.
                taps_e = [(d0, kd, kh) for kd in (0, 2) for kh in range(3)] + [
                    (d1, 1, kh) for kh in range(3)
                ]
                taps_o = [(d1, kd, kh) for kd in (0, 2) for kh in range(3)] + [
                    (d0, 1, kh) for kh in range(3)
                ]
                for hh in (0, 1):
                    ps = {
                        nm: psum.tile([128, NCOL], f32, tag="ps",
                                      name=f"ps_{nm}_{m}{hh}")
                        for nm in ("main0", "main1", "cross0", "cross1")
                    }
                    bank_for = {
                        (0, d0): ps["main0"], (0, d1): ps["cross1"],
                        (1, d1): ps["main1"], (1, d0): ps["cross0"],
                    }
                    for i in range(9):
                        for half, taps in ((0, taps_e), (1, taps_o)):
                            dslab, kd, kh = taps[i]
                            r = kd * 3 + kh
                            p = dslab + kd
                            pst = bank_for[(half, dslab)]
                            first = i == 0 or i == 6
                            last = i == 5 or i == 8
                            for b in range(4):
                                base, rhs = tap_rhs(p, kh, hh, b)
                                nc.tensor.matmul(
                                    pst[32 * b : 32 * b + 32, :],
                                    lhsT=w_sb[base : base + 48, r, :],
                                    rhs=rhs,
                                    start=first,
                                    stop=last,
                                    tile_position=(base, 32 * b),
                                    skip_group_check=(b > 0),
                                )
                    for dd, mn, cx in ((d0, "main0", "cross0"),
                                       (d1, "main1", "cross1")):
                        t = dd * 2 + hh
                        ycs = y_all[:, t * NCOL : (t + 1) * NCOL]
                        # DVE may read only one PSUM operand per instruction:
                        # ScalarE lands the cross bank in SBUF, DVE adds main.
                        nc.scalar.copy(out=ycs, in_=ps[cx][:, :])
                        nc.vector.tensor_add(ycs, ps[mn][:, :], ycs)
                        nc.vector.bn_stats(out=stats_all[:, t, :], in_=ycs)

             # ---- global BN stats ----
             # all partitions hold equal counts, so ship per-partition
             # (mean, mean^2+var); the 1/32 at the end turns the 32-fold sum
             # into the global mean / E[x^2].
             mv = small.tile([128, 2], f32)
             nc.vector.bn_aggr(out=mv, in_=stats_all[:, :, :])
             e2 = small.tile([128, 1], f32)
             nc.vector.tensor_scalar(
                 out=e2,
                 in0=mv[:, 0:1],
                 scalar1=mv[:, 0:1],
                 scalar2=mv[:, 1:2],
                 op0=mybir.AluOpType.mult,
                 op1=mybir.AluOpType.add,
             )

             cin_t = dram.tile([128, 2], f32)
             nc.sync.dma_start(out=cin_t[:, 0:1], in_=mv[:, 0:1])
             nc.sync.dma_start(out=cin_t[:, 1:2], in_=e2)
             cout_t = dram.tile([128, 2], f32)
             for _repc in range(rc):
                 CC_ENG.collective_compute(
                     "AllReduce",
                     mybir.AluOpType.add,
                     replica_groups=[list(range(NCORES))],
                     ins=[cin_t.opt()],
                     outs=[cout_t.opt()],
                 )
             # fetch as [(b',co), j, b], b' a broadcast copy: the innermost
             # reduce sums the 4 h-blocks on all 128 lanes
             gsum = small.tile([128, 2, 4], f32)
             car = cout_t[:, :]
             for rep in range(4):
                 nc.sync.dma_start(
                     out=gsum[32 * rep : 32 * rep + 32, :, :],
                     in_=bass.AP(
                         tensor=car.tensor,
                         offset=car.offset,
                         ap=[[2, COUT], [1, 2], [2 * COUT, 4]],
                     ),
                 )
             red = small.tile([128, 2], f32)
             nc.vector.tensor_reduce(
                 out=red, in_=gsum, axis=mybir.AxisListType.X, op=mybir.AluOpType.add
             )
             sc = small.tile([128, 2], f32)
             nc.vector.tensor_scalar_mul(sc, red[:, :], 1.0 / 32.0)
             mean_t = sc[:, 0:1]
             # nvar = mean^2 - E[x^2] = -var; Ln's scale=-1 flips it back
             nvar_t = small.tile([128, 1], f32)
             nc.vector.tensor_scalar(
                 out=nvar_t,
                 in0=mean_t,
                 scalar1=mean_t,
                 scalar2=sc[:, 1:2],
                 op0=mybir.AluOpType.mult,
                 op1=mybir.AluOpType.subtract,
             )
             # istd = exp(-0.5 * ln(-nvar + eps))
             lnv = small.tile([128, 1], f32)
             nc.scalar.activation(
                 out=lnv,
                 in_=nvar_t,
                 func=mybir.ActivationFunctionType.Ln,
                 bias=eps_sb[:, 0:1],
                 scale=-1.0,
             )
             istd = small.tile([128, 1], f32)
             nc.scalar.activation(
                 out=istd, in_=lnv, func=mybir.ActivationFunctionType.Exp, scale=-0.5
             )
             ab_sb = small.tile([128, 2], f32)
             nc.vector.tensor_mul(ab_sb[:, 0:1], istd, gb_sb[:, 0:1])
             ma_t = small.tile([128, 1], f32)
             nc.vector.tensor_mul(ma_t, mean_t, ab_sb[:, 0:1])
             nc.vector.tensor_scalar(
                 out=ab_sb[:, 1:2],
                 in0=ma_t,
                 scalar1=-1.0,
                 scalar2=gb_sb[:, 1:2],
                 op0=mybir.AluOpType.mult,
                 op1=mybir.AluOpType.add,
             )

             # ---- phase 2: softplus(a*y + b), store fp16 ----
             yh = y_d[:, :, :, :, :, :]
             CH = ch_tiles * NCOL
             for _rep2 in range(r2):
               for c in range(NTILES // ch_tiles):
                 cs = slice(c * CH, (c + 1) * CH)
                 nc.scalar.activation(
                     out=y_all[:, cs],
                     in_=y_all[:, cs],
                     func=mybir.ActivationFunctionType.Exp,
                     scale=ab_sb[:, 0:1],
                     bias=ab_sb[:, 1:2],
                 )
                 nc.scalar.activation(
                     out=y16[:, cs],
                     in_=y_all[:, cs],
                     func=mybir.ActivationFunctionType.Ln,
                     bias=1.0,
                 )
                 # DRAM [t][b][co][512]; partition (b,co) contiguous 512 fp16
                 nc.sync.dma_start(
                     out=bass.AP(
                         tensor=yh.tensor,
                         offset=c * ch_tiles * 65536,
                         ap=[[COUT * NCOL, 4], [NCOL, COUT], [65536, ch_tiles], [1, NCOL]],
                     ),
                     in_=y16[:, cs].rearrange("p (t n) -> p t n", n=NCOL),
                 )
    nc.finalize()
    return nc


_PROGRAM = None


def _get_program():
    global _PROGRAM
    if _PROGRAM is None:
        _PROGRAM = build_program()
    return _PROGRAM


_RUNNER = None


def _get_runner():
    """Compile once; per call feed fresh inputs. Mirrors
    bass2jax.run_bass_via_pjrt's multi-core path without output-buffer
    donation so the jitted executable is reusable across calls."""
    global _RUNNER
    if _RUNNER is not None:
        return _RUNNER
    import jax
    from concourse import bass2jax
    from concourse.bass2jax import _bass_exec_p, partition_id_tensor
    from jax.sharding import Mesh, PartitionSpec
    from jax.experimental.shard_map import shard_map

    bass2jax.install_neuronx_cc_hook()
    nc = _get_program()
    partition_name = nc.partition_id_tensor.name if nc.partition_id_tensor else None
    in_names, out_names, out_avals, zero_outs = [], [], [], []
    for alloc in nc.m.functions[0].allocations:
        if not isinstance(alloc, mybir.MemoryLocationSet):
            continue
        name = alloc.memorylocations[0].name
        if alloc.kind == "ExternalInput":
            if name != partition_name:
                in_names.append(name)
        elif alloc.kind == "ExternalOutput":
            aval = jax.core.ShapedArray(
                tuple(alloc.tensor_shape), mybir.dt.np(alloc.dtype)
            )
            out_names.append(name)
            out_avals.append(aval)
            zero_outs.append(np.zeros(aval.shape, aval.dtype))

    n_params = len(in_names)
    bind_names = list(in_names) + list(out_names)
    if partition_name is not None:
        bind_names.append(partition_name)

    def _body(*args):
        operands = list(args)
        if partition_name is not None:
            operands.append(partition_id_tensor())
        outs = _bass_exec_p.bind(
            *operands,
            out_avals=tuple(out_avals),
            in_names=tuple(bind_names),
            out_names=tuple(out_names),
            lowering_input_output_aliases=(),
            sim_require_finite=True,
            sim_require_nnan=True,
            nc=nc,
        )
        return tuple(outs)

    devices = jax.devices()[:NCORES]
    mesh = Mesh(np.asarray(devices), ("core",))
    in_specs = (PartitionSpec("core"),) * (n_params + len(out_names))
    out_specs = (PartitionSpec("core"),) * len(out_names)
    sharded = jax.jit(
        shard_map(_body, mesh=mesh, in_specs=in_specs, out_specs=out_specs,
                  check_rep=False),
        keep_unused=True,
    )
    concat_zero = [
        np.zeros((NCORES * z.shape[0], *z.shape[1:]), z.dtype) for z in zero_outs
    ]

    def run(in_maps):
        concat_in = [
            np.concatenate([np.asarray(in_maps[c][name]) for c in range(NCORES)],
                           axis=0)
            for name in in_names
        ]
        out_arrs = sharded(*concat_in, *concat_zero)
        fetched = [
            np.asarray(a).reshape(NCORES, *out_avals[i].shape)
            for i, a in enumerate(out_arrs)
        ]
        return [
            {name: fetched[i][c] for i, name in enumerate(out_names)}
            for c in range(NCORES)
        ]

    _RUNNER = run
    return run


def make_inputs(x, weight, gamma, beta):
    w = preprocess_weights(weight)
    w9 = build_w9(w).astype(np.float16)
    gb = np.stack([gamma.astype(np.float32), beta.astype(np.float32)], 0)
    x = np.asarray(x, np.float32)
    in_maps = []
    for c in range(NCORES):
        n, dh = c // 2, c % 2
        d0 = dh * DSH
        xp = np.zeros((CIN, NPLANES, HP, WP), np.float32)
        lo, hi = d0 - 1, d0 + DSH + 1
        slo, shi = max(lo, 0), min(hi, D)
        xp[:, slo - lo : shi - lo, 1 : H + 1, 1 : W + 1] = x[n, :, slo:shi]
        in_maps.append({"xs": build_xs(xp), "w9": w9, "gb": gb})
    return in_maps


def kernel(x, weight, bias, gamma, beta):
    run = _get_runner()
    in_maps = make_inputs(x, weight, gamma, beta)
    results = run(in_maps)
    out = np.empty((N, COUT, D, H, W), np.float32)
    for c in range(NCORES):
        n, dh = c // 2, c % 2
        yc = results[c]["y"].astype(np.float32)  # [DSH, 2, 4, COUT, 8, W]
        yc = yc.transpose(3, 0, 1, 2, 4, 5).reshape(COUT, DSH, H, W)
        out[n, :, dh * DSH : (dh + 1) * DSH] = yc
    return out
